# revision 29
# baseline (speedup 1.0000x reference)
"""DeFeat distillation loss on 8 Trainium2 NeuronCores (Bass/Tile).

Data-parallel over the batch dim (B=8 -> 1 batch element per core).
Per core, features are viewed as [C=256, H*W] and streamed in column
blocks (levels 2-4 are fused into one multi-segment block so the DMA
stream never drains on the small levels):
  psum    = feat_t                       [DVE/ACT copy seed, alternating]
  psum   += (-W) @ feat_s                [TensorE fp32r, accumulate onto
                                          seeded values via a one-time
                                          has_written warmup matmul pass]
  rs_tot[slot] = rowsum((psum - b)^2), dd = (psum-b)^2 (bf16)  [ScalarE]
  rs_gt[slot]  = rowsum(dd * mask)       [VectorE fused mult+accum]
Masks are rasterized on host (<90KB/core, bf16) and broadcast across
partitions on GpSimd. The host sums the [128, n_slots] partials across
cores/slots per level, applies sqrt and the gt/bg weights.
"""

import os
import sys

for _p in ("/opt/trn_rl_repo", os.path.expanduser("~/.axon_site/_ro/trn_rl_repo")):
    if os.path.isdir(_p) and _p not in sys.path:
        sys.path.insert(0, _p)

import numpy as np

WEIGHT_GT = 0.004
WEIGHT_BG = 0.0002
STRIDES = (8, 16, 32, 64, 128)
SIZES = (128, 64, 32, 16, 8)
HWS = tuple(s * s for s in SIZES)          # (16384, 4096, 1024, 256, 64)
B, C, NBOX = 8, 256, 16
N_CORES = 8
TILE_N = 512                               # matmul free-dim tile
PS_W = 1024                                # psum sub-unit width (2 banks)
N_LEVELS = 5
MASK_LEN = sum(HWS)                        # 21824
MASK_OFF = tuple(sum(HWS[:i]) for i in range(N_LEVELS))

# Seed PSUM with feat_t via engine copy, accumulate -W@s on top (needs the
# has_written warmup matmuls). False falls back to an in-place subtract.
USE_ACCUM_SEED = True

# Blocks: list of segment lists [(lvl, c0, w), ...]. First/last blocks are
# narrow so compute starts early and finishes quickly; the small levels sit
# mid-stream fused into one block (their mask offsets are contiguous).
BLOCKS = [
    [(0, 0, 1024)], [(0, 1024, 1024)],
    [(0, 2048, 2048)], [(0, 4096, 2048)], [(0, 6144, 2048)],
    [(2, 0, 1024), (3, 0, 256), (4, 0, 64)],
    [(1, 0, 2048)], [(1, 2048, 2048)],
    [(0, 8192, 2048)], [(0, 10240, 2048)], [(0, 12288, 2048)],
    [(0, 14336, 1024)], [(0, 15360, 1024)],
]


def _block_layout():
    """Derive per-block column layout, sub-units, and slot tables."""
    blocks = []
    gt_slots = {l: [] for l in range(N_LEVELS)}
    tot_slots = {l: [] for l in range(N_LEVELS)}
    n_gt = 0
    n_tot = 0
    for segs in BLOCKS:
        w_blk = sum(w for (_, _, w) in segs)
        # segment -> block-local column range
        seg_cols = []
        col = 0
        for (lvl, c0, w) in segs:
            seg_cols.append((lvl, c0, col, w))
            col += w
        # mask contiguity check
        m0 = MASK_OFF[segs[0][0]] + segs[0][1]
        mc = m0
        for (lvl, c0, w) in segs:
            assert MASK_OFF[lvl] + c0 == mc, "mask segments not contiguous"
            mc += w
        # matmul tiles: (block_col, n, lvl)
        mm_tiles = []
        for (lvl, c0, bcol, w) in seg_cols:
            for j in range(0, w, TILE_N):
                mm_tiles.append((bcol + j, min(TILE_N, w - j), lvl))
        # per-oc structures are identical; store one copy
        subunits = []
        for a in range(0, w_blk, PS_W):
            b_ = min(a + PS_W, w_blk)
            # ACT pieces: sub-unit range intersected with segments
            pieces = []
            for (lvl, c0, bcol, w) in seg_cols:
                lo = max(a, bcol)
                hi = min(b_, bcol + w)
                if lo < hi:
                    pieces.append((lo, hi, lvl))
            subunits.append((a, b_, pieces))
        blocks.append(dict(segs=seg_cols, w_blk=w_blk, mask0=m0,
                           mm_tiles=mm_tiles, subunits=subunits))
    # slot numbering must match emission order below: loop blocks, oc, ...
    for blk in blocks:
        blk["tot_slot"] = {}
        blk["gt_slot"] = {}
        for oc in range(2):
            for (a, b_, pieces) in blk["subunits"]:
                for (lo, hi, lvl) in pieces:
                    blk["tot_slot"][(oc, lo, hi)] = n_tot
                    tot_slots[lvl].append(n_tot)
                    n_tot += 1
            for (lvl, c0, bcol, w) in blk["segs"]:
                blk["gt_slot"][(oc, bcol)] = n_gt
                gt_slots[lvl].append(n_gt)
                n_gt += 1
    return blocks, tot_slots, gt_slots, n_tot, n_gt


BLOCK_LAYOUT, TOT_SLOTS, GT_SLOTS, N_TOT, N_GT = _block_layout()
MAX_BW = max(b["w_blk"] for b in BLOCK_LAYOUT)


def _build_module():
    import concourse.mybir as mybir
    from concourse import bacc
    from concourse.tile import TileContext

    dt = mybir.dt
    nc = bacc.Bacc("TRN2", target_bir_lowering=False, debug=False,
                   num_devices=N_CORES)

    fs = [nc.dram_tensor(f"fs{l}", [C, HWS[l]], dt.float32, kind="ExternalInput")
          for l in range(N_LEVELS)]
    ft = [nc.dram_tensor(f"ft{l}", [C, HWS[l]], dt.float32, kind="ExternalInput")
          for l in range(N_LEVELS)]
    # 20 weight chunks + 1 zeros chunk (has_written warmup lhsT)
    wt_d = nc.dram_tensor("wt", [128, (N_LEVELS * 4 + 1) * 128], dt.float32,
                          kind="ExternalInput")
    bias_d = nc.dram_tensor("bias", [128, N_LEVELS * 2], dt.float32,
                            kind="ExternalInput")
    mask_d = nc.dram_tensor("mask", [1, MASK_LEN], dt.bfloat16,
                            kind="ExternalInput")
    out_tot_d = nc.dram_tensor("out_tot", [128, N_TOT], dt.float32,
                               kind="ExternalOutput")
    out_gt_d = nc.dram_tensor("out_gt", [128, N_GT], dt.float32,
                              kind="ExternalOutput")

    f32r = dt.float32r
    SUB = mybir.AluOpType.subtract
    MUL = mybir.AluOpType.mult
    SQUARE = mybir.ActivationFunctionType.Square

    with TileContext(nc) as tc:
        with (
            tc.tile_pool(name="const", bufs=1) as const_pool,
            tc.tile_pool(name="feat", bufs=3) as feat_pool,
            tc.tile_pool(name="maskp", bufs=2) as mask_pool,
            tc.tile_pool(name="work", bufs=3) as work_pool,
            tc.tile_pool(name="acc", bufs=1) as acc_pool,
            tc.tile_pool(name="ps", bufs=1, space="PSUM") as psum_pool,
        ):
            wt = const_pool.tile([128, (N_LEVELS * 4 + 1) * 128], f32r)
            bias = const_pool.tile([128, N_LEVELS * 2], dt.float32)

            rs_tot = acc_pool.tile([128, N_TOT], dt.float32)
            rs_gt = acc_pool.tile([128, N_GT], dt.float32)
            nc.vector.memset(rs_tot[:], 0.0)
            nc.vector.memset(rs_gt[:], 0.0)

            # wt layout: [zeros(128) | L0 weights(512) | L1..L4 weights].
            # The small first DMA unblocks the warmup matmuls and the
            # level-0 blocks while the big feature streams ramp up.
            nc.sync.dma_start(out=wt[:, 0:640],
                              in_=wt_d[:, 0:640].bitcast(f32r))
            zero_w = wt[:, 0:128]

            # four persistent 2-bank psum tiles, sub-units rotate through them
            ps_pool = [psum_pool.tile([128, PS_W], dt.float32, tag=f"ps{i}",
                                      name=f"ps{i}") for i in range(4)]
            if USE_ACCUM_SEED:
                # set every has_written bit once; later matmuls accumulate
                # (start=False) onto engine-seeded values
                for pstile in ps_pool:
                    for j in range(0, PS_W, TILE_N):
                        nc.tensor.matmul(pstile[:, j:j + TILE_N],
                                         zero_w, wt[:, 0:TILE_N],
                                         start=True, stop=True)

            su_idx = 0
            first = True
            for blk in BLOCK_LAYOUT:
                w_blk = blk["w_blk"]
                s_lo = feat_pool.tile([128, MAX_BW], f32r, tag="s_lo")
                s_hi = feat_pool.tile([128, MAX_BW], f32r, tag="s_hi")
                t_lo = feat_pool.tile([128, MAX_BW], dt.float32, tag="t_lo")
                t_hi = feat_pool.tile([128, MAX_BW], dt.float32, tag="t_hi")
                for (lvl, c0, bcol, w) in blk["segs"]:
                    nc.sync.dma_start(
                        out=s_lo[:, bcol:bcol + w],
                        in_=fs[lvl][0:128, c0:c0 + w].bitcast(f32r))
                    nc.sync.dma_start(
                        out=s_hi[:, bcol:bcol + w],
                        in_=fs[lvl][128:256, c0:c0 + w].bitcast(f32r))
                    nc.sync.dma_start(
                        out=t_lo[:, bcol:bcol + w],
                        in_=ft[lvl][0:128, c0:c0 + w])
                    nc.sync.dma_start(
                        out=t_hi[:, bcol:bcol + w],
                        in_=ft[lvl][128:256, c0:c0 + w])

                if first:
                    # remaining constants behind the first feature block
                    nc.sync.dma_start(
                        out=wt[:, 640:(N_LEVELS * 4 + 1) * 128],
                        in_=wt_d[:, 640:(N_LEVELS * 4 + 1) * 128].bitcast(f32r))
                    nc.sync.dma_start(out=bias[:], in_=bias_d[:])
                    first = False

                m_row = mask_pool.tile([1, MAX_BW], dt.bfloat16, tag="m_row")
                nc.sync.dma_start(
                    out=m_row[:, :w_blk],
                    in_=mask_d[0:1, blk["mask0"]:blk["mask0"] + w_blk])
                mb = mask_pool.tile([128, MAX_BW], dt.bfloat16, tag="mb")
                nc.gpsimd.partition_broadcast(mb[:, :w_blk], m_row[:, :w_blk],
                                              channels=128)

                t_chunks = (t_lo, t_hi)
                s_chunks = (s_lo, s_hi)
                for oc in range(2):
                    dd_blk = work_pool.tile([128, MAX_BW], dt.bfloat16,
                                            tag="dd")
                    for (a, b_, pieces) in blk["subunits"]:
                        ps = ps_pool[su_idx % 4]
                        hw_ = b_ - a
                        if USE_ACCUM_SEED:
                            # psum = t, then += (-W)@s (W negated on host)
                            if su_idx % 2 == 0:
                                nc.vector.tensor_copy(
                                    ps[:, :hw_], t_chunks[oc][:, a:b_])
                            else:
                                nc.scalar.copy(
                                    ps[:, :hw_], t_chunks[oc][:, a:b_])
                        for (bcol, n, lvl) in blk["mm_tiles"]:
                            if not (a <= bcol < b_):
                                continue
                            widx = (lvl * 2 + oc) * 2
                            po = bcol - a
                            if USE_ACCUM_SEED:
                                nc.tensor.matmul(
                                    ps[:, po:po + n],
                                    wt[:, (widx + 1) * 128:(widx + 2) * 128],
                                    s_lo[:, bcol:bcol + n],
                                    start=False, stop=False,
                                    skip_group_check=True)
                                nc.tensor.matmul(
                                    ps[:, po:po + n],
                                    wt[:, (widx + 2) * 128:(widx + 3) * 128],
                                    s_hi[:, bcol:bcol + n],
                                    start=False, stop=True,
                                    skip_group_check=True)
                            else:
                                nc.tensor.matmul(
                                    ps[:, po:po + n],
                                    wt[:, (widx + 1) * 128:(widx + 2) * 128],
                                    s_lo[:, bcol:bcol + n],
                                    start=True, stop=False)
                                nc.tensor.matmul(
                                    ps[:, po:po + n],
                                    wt[:, (widx + 2) * 128:(widx + 3) * 128],
                                    s_hi[:, bcol:bcol + n],
                                    start=False, stop=True)
                        for (lo, hi, lvl) in pieces:
                            bias_ap = bias[:, lvl * 2 + oc:lvl * 2 + oc + 1]
                            if not USE_ACCUM_SEED:
                                nc.vector.scalar_tensor_tensor(
                                    ps[:, lo - a:hi - a],
                                    t_chunks[oc][:, lo:hi], bias_ap,
                                    ps[:, lo - a:hi - a], op0=SUB, op1=SUB)
                            tslot = blk["tot_slot"][(oc, lo, hi)]
                            nc.scalar.activation(
                                dd_blk[:, lo:hi], ps[:, lo - a:hi - a], SQUARE,
                                bias=bias_ap if USE_ACCUM_SEED else 0.0,
                                accum_out=rs_tot[:, tslot:tslot + 1])
                        su_idx += 1
                    scr_blk = work_pool.tile([128, MAX_BW], dt.bfloat16,
                                             tag="scr")
                    for (lvl, c0, bcol, w) in blk["segs"]:
                        gslot = blk["gt_slot"][(oc, bcol)]
                        nc.vector.scalar_tensor_tensor(
                            scr_blk[:, bcol:bcol + w],
                            dd_blk[:, bcol:bcol + w],
                            1.0,
                            mb[:, bcol:bcol + w],
                            op0=MUL, op1=MUL,
                            accum_out=rs_gt[:, gslot:gslot + 1])

            nc.sync.dma_start(out=out_tot_d[:], in_=rs_tot[:])
            nc.sync.dma_start(out=out_gt_d[:], in_=rs_gt[:])

    nc.compile()
    return nc


def _rasterize_masks(gt_bboxes):
    """Host-side mask rasterization, mirroring reference.gt_mask in fp32.

    Returns [B, MASK_LEN] float32 (per-level masks concatenated)."""
    out = np.zeros((B, MASK_LEN), np.float32)
    for lvl in range(N_LEVELS):
        h = w = SIZES[lvl]
        stride = np.float32(STRIDES[lvl])
        off = MASK_OFF[lvl]
        q = np.floor(gt_bboxes.astype(np.float32) / stride).astype(np.int32)
        lx = np.minimum(q[..., 0], w - 1)
        ly = np.minimum(q[..., 1], h - 1)
        rx = np.minimum(q[..., 2], w - 1)
        ry = np.minimum(q[..., 3], h - 1)
        for b in range(B):
            m = np.zeros((h, w), bool)
            for i in range(gt_bboxes.shape[1]):
                if lx[b, i] == rx[b, i] or ly[b, i] == ry[b, i]:
                    m[ly[b, i], lx[b, i]] = True
                else:
                    m[ly[b, i]:ry[b, i], lx[b, i]:rx[b, i]] = True
            out[b, off:off + h * w] = m.reshape(-1).astype(np.float32)
    return out


_NC_CACHE = None


def _get_nc():
    global _NC_CACHE
    if _NC_CACHE is None:
        _NC_CACHE = _build_module()
    return _NC_CACHE


def _run(in_maps, trace=False, trace_cores=None):
    from concourse.bass_utils import run_bass_kernel_spmd

    kwargs = {}
    if trace:
        kwargs.update(trace=True, trace_cores=trace_cores or [0])
    return run_bass_kernel_spmd(_get_nc(), in_maps, core_ids=list(range(N_CORES)),
                                **kwargs)


def _pack_const(inputs):
    """Pack replicated weights/bias: cols [0,128) are zeros (warmup lhsT),
    then chunk ((lvl*2+oc)*2+kc) at 128+idx*128 holds
    w_lvl[oc*128+o_local, kc*128+c_local] transposed.
    Negated under USE_ACCUM_SEED: psum = t + (-W)@s; bias likewise."""
    sgn = np.float32(-1.0 if USE_ACCUM_SEED else 1.0)
    wt_packed = np.zeros((128, (N_LEVELS * 4 + 1) * 128), np.float32)
    bias_packed = np.zeros((128, N_LEVELS * 2), np.float32)
    for lvl in range(N_LEVELS):
        w = np.asarray(inputs[f"adapt_w{lvl}"], np.float32)
        bvec = np.asarray(inputs[f"adapt_b{lvl}"], np.float32)
        for oc in range(2):
            bias_packed[:, lvl * 2 + oc] = sgn * bvec[oc * 128:(oc + 1) * 128]
            for kc in range(2):
                idx = (lvl * 2 + oc) * 2 + kc
                blk = w[oc * 128:(oc + 1) * 128, kc * 128:(kc + 1) * 128]
                wt_packed[:, 128 + idx * 128:128 + (idx + 1) * 128] = sgn * blk.T
    return wt_packed, bias_packed


def kernel(_trace=False, _return_results=False, **inputs):
    import ml_dtypes

    gt_bboxes = np.asarray(inputs["gt_bboxes"], np.float32)
    masks = _rasterize_masks(gt_bboxes).astype(ml_dtypes.bfloat16)
    wt_packed, bias_packed = _pack_const(inputs)

    in_maps = []
    for b in range(N_CORES):
        m = {"wt": wt_packed, "bias": bias_packed,
             "mask": masks[b:b + 1]}
        for lvl in range(N_LEVELS):
            m[f"fs{lvl}"] = np.ascontiguousarray(
                np.asarray(inputs[f"feat_s{lvl}"][b], np.float32).reshape(C, HWS[lvl]))
            m[f"ft{lvl}"] = np.ascontiguousarray(
                np.asarray(inputs[f"feat_t{lvl}"][b], np.float32).reshape(C, HWS[lvl]))
        in_maps.append(m)

    res = _run(in_maps, trace=_trace)

    loss = np.float64(0.0)
    for lvl in range(N_LEVELS):
        s_tot = np.float64(0.0)
        s_gt = np.float64(0.0)
        for c in range(N_CORES):
            s_tot += res.results[c]["out_tot"][:, TOT_SLOTS[lvl]].astype(np.float64).sum()
            s_gt += res.results[c]["out_gt"][:, GT_SLOTS[lvl]].astype(np.float64).sum()
        s_bg = s_tot - s_gt
        loss += WEIGHT_GT * np.sqrt(s_gt + 1e-8) + WEIGHT_BG * np.sqrt(s_bg + 1e-8)

    out = np.array(loss, dtype=np.float32)
    if _return_results:
        return out, res
    return out
